# revision 8
# baseline (speedup 1.0000x reference)
"""LMU (Legendre Memory Unit) RNN kernel for Trainium2, 8 NeuronCores.

Strategy
--------
Data-parallel over batch: each of the 8 cores runs B_local = 16 sequences
through the full T=784-step recurrence; outputs are concatenated on host.

Per-step math is algebraically folded (host, float64).  With
u_t = e_x x_t + h_t e_h^T + m_t e_m^T, Ad = I + AT, w = W_m @ BT:

    m_{t+1} = Amm m_t + BT a_t + wx_m x_t,   a_t := e_h . h_t  (scalar/seq)
    pre_h   = Wq h_t + Wp m_t + wx_h x_t,    h_{t+1} = tanh(pre_h)

where Amm = Ad + BT e_m, Wq = W_h + w e_h, Wp = W_m Ad + w e_m.  The
m<-h coupling (BT e_h) is RANK-1, so h-rounds never write the m psum:
they emit only the 1024 pre_h columns plus one extra column a_t, and the
BT a_t term is injected one step later as a rank-1 "a-mm" using the
stored state m' := m - BT a_prev (compensations (Wp BT) a_prev and
(Amm BT) a_prev are folded into the a-mms; exact algebra, see
check_math.py).

On-chip schedule.  Per-core batch 16 -> state K-tiles [128, 16]
stationary, weights streamed, 4 column groups (tile_position=(0,32j))
concurrent.  Two psum tiles: psh [128, 257+] (pre_h + a col; written by
x/a/m/h rounds) and psm [128, 128] (m'; written by x/a/m rounds ONLY,
so it completes ~900ns before the step ends and its CAST+transpose
chain hides under the h-rounds).  Separate tiles also decouple the
tile-framework's reader chaining (ACT on psh vs CAST on psm).

The psum->state transposition is the DVE 32x32 block transpose; the
feature permutation makes block semantics line up: psum[32j+b, 32k+q] ->
st[32j+q, 32k+b], i.e. psum slot (strip j, col 32k+q) IS state K-tile k,
partition 32j+q.  Batch (16) < 32 so transposed batch occupies cols
32k..32k+16; slack cols hold transposed psum gap rows (zeros: one-time
psum memset; matmuls only write partitions 32j..32j+16).

Per step:  PE: x-mms (K=1 rank-1, start=True) -> m-rounds (4, fed by
stM) -> a-mms (K=1, fed by a_sb) -> h-rounds (8 x N=257, fed by stH).
DVE: CAST psm->fp16, T1 transpose -> stM; CAST_a psh a-block -> fp16,
T_a transpose -> a_sb; T2 transpose tanh output -> stH.
ACT: tanh psh[:,0:256] -> hs (issued after CAST_a: psh readers chain in
issue order).

x is stored compactly ([1, 16] per step on partition 0); two half-body
x tiles are DMA-double-buffered so the per-body refill never stalls.
"""

import numpy as np

import concourse.bass as bass
import concourse.mybir as mybir
import concourse.tile as tile
from concourse import bacc
from concourse.bass import ds, ts
from concourse.bass_utils import run_bass_kernel_spmd

T, B, UNITS, ORDER = 784, 128, 1024, 512
NCORES = 8
BL = B // NCORES          # 16 sequences per core
SOUT = UNITS + ORDER
KT = 12                   # state K-tiles of 128 (8 h + 4 m)
NSTRIP = 4                # PE column groups
NWH = 257                 # psh cols per strip: 256 pre_h + 1 a (strip 0)
NWM = 128                 # psm cols per strip
PSHW = 288                # psh tile width (a-block padded to 32)
UNROLL = 112              # steps per For_i body (two x half-tiles)
XH = UNROLL // 2

FP16 = mybir.dt.float16
FP32 = mybir.dt.float32


def _perms():
    # psum slot -> feature: psh (j, c<256) = h feature 256j+c;
    # psm (j, c) = m feature 128j+c (+1024 globally)
    # state: (K-tile r, partition 32j+q) -> feature
    IP = np.empty(SOUT, np.int64)
    for r in range(KT):
        for j in range(NSTRIP):
            for q in range(32):
                IP[r * 128 + 32 * j + q] = (
                    256 * j + 32 * r + q if r < 8 else 1024 + 128 * j + 32 * (r - 8) + q
                )
    return IP


def _build_weights(e_x, e_h, e_m, W_x, W_h, W_m, AT, BT):
    """Host-side fold into the phase-2' arrays (float64 -> fp16)."""
    f = np.float64
    e_x, e_h, e_m = e_x.astype(f), e_h.astype(f), e_m.astype(f)
    W_x, W_h, W_m = W_x.astype(f), W_h.astype(f), W_m.astype(f)
    AT, BT = AT.astype(f), BT.astype(f)
    Ad = np.eye(ORDER) + AT
    w = W_m @ BT                                    # (U, 1)
    Wq = W_h + w @ e_h                              # (U, U)
    Wp = W_m @ Ad + w @ e_m                         # (U, O)
    wx_h = (W_x + w * e_x)[:, 0]                    # (U,)
    Amm = Ad + BT @ e_m                             # (O, O)
    wx_m = (BT * e_x)[:, 0]                         # (O,)
    v_h = (Wp @ BT)[:, 0]                           # (U,) a-comp -> pre_h
    v_m = (Amm @ BT)[:, 0]                          # (O,) a-comp -> m'

    IP = _perms()
    IPh = IP[: 8 * 128]
    IPm = IP[8 * 128 :] - 1024
    OPh = [256 * j + np.arange(256) for j in range(NSTRIP)]
    OPm = [128 * j + np.arange(128) for j in range(NSTRIP)]

    wh = np.zeros((8 * 128, NSTRIP * NWH))
    wmh = np.zeros((4 * 128, NSTRIP * NWH))
    wmm = np.zeros((4 * 128, NSTRIP * NWM))
    xwh = np.zeros((NSTRIP, NWH))
    xwm = np.zeros((NSTRIP, NWM))
    awh = np.zeros((1, NSTRIP * NWH))
    awm = np.zeros((1, NSTRIP * NWM))
    for j in range(NSTRIP):
        ch = slice(j * NWH, j * NWH + 256)
        cm = slice(j * NWM, (j + 1) * NWM)
        wh[:, ch] = Wq[OPh[j]][:, IPh].T
        if j == 0:
            wh[:, j * NWH + 256] = e_h[0, IPh]
        wmh[:, ch] = Wp[OPh[j]][:, IPm].T
        wmm[:, cm] = Amm[OPm[j]][:, IPm].T
        xwh[j, :256] = wx_h[OPh[j]]
        xwm[j, :] = wx_m[OPm[j]]
        awh[0, ch] = v_h[OPh[j]]
        awm[0, cm] = v_m[OPm[j]]
    h16 = np.float16
    return (wh.astype(h16), wmh.astype(h16), wmm.astype(h16),
            xwh.astype(h16), xwm.astype(h16), awh.astype(h16),
            awm.astype(h16))


def _build_nc(t_steps=T, unroll=UNROLL):
    assert t_steps % unroll == 0 and unroll % 2 == 0
    iters = t_steps // unroll
    xh = unroll // 2
    nc = bacc.Bacc("TRN2", target_bir_lowering=False, num_devices=NCORES)

    wh_dram = nc.dram_tensor("wh", [8 * 128, NSTRIP * NWH], FP16,
                             kind="ExternalInput")
    wmh_dram = nc.dram_tensor("wmh", [4 * 128, NSTRIP * NWH], FP16,
                              kind="ExternalInput")
    wmm_dram = nc.dram_tensor("wmm", [4 * 128, NSTRIP * NWM], FP16,
                              kind="ExternalInput")
    xwh_dram = nc.dram_tensor("xwh", [NSTRIP, NWH], FP16,
                              kind="ExternalInput")
    xwm_dram = nc.dram_tensor("xwm", [NSTRIP, NWM], FP16,
                              kind="ExternalInput")
    awh_dram = nc.dram_tensor("awh", [1, NSTRIP * NWH], FP16,
                              kind="ExternalInput")
    awm_dram = nc.dram_tensor("awm", [1, NSTRIP * NWM], FP16,
                              kind="ExternalInput")
    wd_dram = nc.dram_tensor("wd", [UNITS + 1, 10], FP16, kind="ExternalInput")
    x_dram = nc.dram_tensor(
        "xs", [NSTRIP, (t_steps + unroll) * 128], FP16, kind="ExternalInput"
    )
    out_dram = nc.dram_tensor("out", [BL, 10], FP32, kind="ExternalOutput")

    TANH = mybir.ActivationFunctionType.Tanh

    with tile.TileContext(nc) as tc:
        with (
            tc.tile_pool(name="const", bufs=1) as cpool,
            tc.tile_pool(name="state", bufs=1) as spool,
            tc.tile_pool(name="work", bufs=2) as wpool,
            tc.tile_pool(name="psum", bufs=1, space="PSUM") as ppool,
        ):
            # ---- persistent SBUF ----
            wh_sb = cpool.tile([128, 8 * NSTRIP * NWH], FP16, tag="wh_sb")
            for r in range(8):
                nc.sync.dma_start(
                    wh_sb[:, ts(r, NSTRIP * NWH)], wh_dram[ts(r, 128), :]
                )
            wmh_sb = cpool.tile([128, 4 * NSTRIP * NWH], FP16, tag="wmh_sb")
            wmm_sb = cpool.tile([128, 4 * NSTRIP * NWM], FP16, tag="wmm_sb")
            for r in range(4):
                nc.sync.dma_start(
                    wmh_sb[:, ts(r, NSTRIP * NWH)], wmh_dram[ts(r, 128), :]
                )
                nc.sync.dma_start(
                    wmm_sb[:, ts(r, NSTRIP * NWM)], wmm_dram[ts(r, 128), :]
                )
            xwh_sb = cpool.tile([NSTRIP, NWH], FP16, tag="xwh_sb")
            xwm_sb = cpool.tile([NSTRIP, NWM], FP16, tag="xwm_sb")
            awh_sb = cpool.tile([1, NSTRIP * NWH], FP16, tag="awh_sb")
            awm_sb = cpool.tile([1, NSTRIP * NWM], FP16, tag="awm_sb")
            nc.sync.dma_start(xwh_sb[:, :], xwh_dram[:, :])
            nc.sync.dma_start(xwm_sb[:, :], xwm_dram[:, :])
            nc.sync.dma_start(awh_sb[:, :], awh_dram[:, :])
            nc.sync.dma_start(awm_sb[:, :], awm_dram[:, :])
            wd_sb = cpool.tile([128, 8 * 10], FP16, tag="wd_sb")
            for r in range(8):
                nc.sync.dma_start(wd_sb[:, ts(r, 10)], wd_dram[ts(r, 128), :])
            bias_sb = cpool.tile([1, 10], FP16, tag="bias_sb")
            nc.sync.dma_start(bias_sb[:, :], wd_dram[1024:1025, :])
            ones_sb = cpool.tile([1, BL], FP16, tag="ones_sb")
            nc.vector.memset(ones_sb[:, :], 1.0)

            x_stg = [
                spool.tile(
                    [NSTRIP, xh * 128], FP16, tag=f"x_stg{a}", name=f"x_stg{a}"
                )
                for a in range(2)
            ]
            nc.sync.dma_start(x_stg[0][:, :], x_dram[:, 0 : xh * 128])
            nc.sync.dma_start(x_stg[1][:, :], x_dram[:, xh * 128 : unroll * 128])

            # state double buffers: step u reads parity u%2, writes 1-u%2
            stHa = [
                spool.tile([128, 128], FP16, tag=f"stHa{p}", name=f"stHa{p}")
                for p in range(2)
            ]
            stHb = [
                spool.tile([128, 128], FP16, tag=f"stHb{p}", name=f"stHb{p}")
                for p in range(2)
            ]
            stM = [
                spool.tile([128, 128], FP16, tag=f"stM{p}", name=f"stM{p}")
                for p in range(2)
            ]
            a_sb = [
                spool.tile([32, 32], FP16, tag=f"a{p}", name=f"a{p}")
                for p in range(2)
            ]
            msb = [
                spool.tile([128, 128], FP16, tag=f"ms{p}", name=f"ms{p}")
                for p in range(2)
            ]
            asb16 = [
                spool.tile([32, 32], FP16, tag=f"as{p}", name=f"as{p}")
                for p in range(2)
            ]
            hsa = [
                spool.tile([128, 128], FP16, tag=f"hsa{p}", name=f"hsa{p}")
                for p in range(2)
            ]
            hsb = [
                spool.tile([128, 128], FP16, tag=f"hsb{p}", name=f"hsb{p}")
                for p in range(2)
            ]
            for t_ in stHa + stHb + stM + a_sb:
                nc.vector.memset(t_[:, :], 0.0)

            psh = [
                ppool.tile([128, PSHW], FP32, tag=f"psh{p}", name=f"psh{p}")
                for p in range(2)
            ]
            psm = [
                ppool.tile([128, NWM], FP32, tag=f"psm{p}", name=f"psm{p}")
                for p in range(2)
            ]
            # zero once: gap partitions 32j+16..32j+32 (matmuls write only
            # 16 batch rows) and psh cols 257:288 feed the transposed slack
            for t_ in psh + psm:
                nc.vector.memset(t_[:, :], 0.0)

            def step(u):
                p = u % 2
                wp = 1 - p
                ph, pm = psh[p], psm[p]
                xs = x_stg[0] if u < xh else x_stg[1]
                xoff = (u % xh) * 128
                # x rounds: K=4 host-built spread (row k carries x at cols
                # 32k..32k+16, rhs row k = strip k weights); one mm per
                # psum tile, M=128 start=True clears gaps too
                xsl = xs[:, ds(xoff, 128)]
                nc.tensor.matmul(
                    ph[:, 0:NWH], xsl, xwh_sb[:, :],
                    start=True, stop=False,
                )
                nc.tensor.matmul(
                    pm[:, :], xsl, xwm_sb[:, :],
                    start=True, stop=False,
                )
                # m rounds (state m', 4 K-tiles) -> both psums
                for r in range(4):
                    lhsT = stM[p][:, ds(32 * r, BL)]
                    for j in range(NSTRIP):
                        nc.tensor.matmul(
                            ph[ds(32 * j, BL), 0:NWH], lhsT,
                            wmh_sb[:, ds(r * NSTRIP * NWH + j * NWH, NWH)],
                            start=False, stop=False, tile_position=(0, 32 * j),
                        )
                        nc.tensor.matmul(
                            pm[ds(32 * j, BL), :], lhsT,
                            wmm_sb[:, ds(r * NSTRIP * NWM + j * NWM, NWM)],
                            start=False, stop=False, tile_position=(0, 32 * j),
                        )
                # a-mms: rank-1 compensation with a(t-1); last psm writer
                for j in range(NSTRIP):
                    asl = a_sb[p][0:1, 0:BL]
                    nc.tensor.matmul(
                        ph[ds(32 * j, BL), 0:NWH], asl,
                        awh_sb[0:1, ds(j * NWH, NWH)],
                        start=False, stop=False, tile_position=(0, 32 * j),
                    )
                    nc.tensor.matmul(
                        pm[ds(32 * j, BL), :], asl,
                        awm_sb[0:1, ds(j * NWM, NWM)],
                        start=False, stop=True, tile_position=(0, 32 * j),
                    )
                # h rounds (8 K-tiles, N=257); last psh writer
                for r in range(8):
                    lhsT = (stHa[p][:, ds(32 * r, BL)] if r < 4
                            else stHb[p][:, ds(32 * (r - 4), BL)])
                    for j in range(NSTRIP):
                        nc.tensor.matmul(
                            ph[ds(32 * j, BL), 0:NWH], lhsT,
                            wh_sb[:, ds(r * NSTRIP * NWH + j * NWH, NWH)],
                            start=False, stop=(r == 7),
                            tile_position=(0, 32 * j),
                        )
                # finalize.  DVE: m-chain first (psm completed early, its
                # chain hides under the h-rounds), then the a-block
                # (CAST_a issued before ACT: psh readers chain in issue
                # order), then T2 after tanh.
                # ACT first: psh readers chain in issue order, and the
                # tanh->T2a->h-rounds path is the critical chain; the
                # a-extraction has ~1.2us of slack (a-mms run ~900ns into
                # the NEXT step)
                nc.vector.tensor_copy(msb[p][:, :], pm[:, :])
                nc.vector.transpose(stM[wp][:, :], msb[p][:, :])
                nc.scalar.activation(hsa[p][:, :], ph[:, 0:128], TANH)
                nc.scalar.activation(hsb[p][:, :], ph[:, 128:256], TANH)
                nc.vector.transpose(stHa[wp][:, :], hsa[p][:, :])
                nc.vector.transpose(stHb[wp][:, :], hsb[p][:, :])
                nc.vector.tensor_copy(asb16[p][:, :], ph[0:32, 256:288])
                nc.vector.transpose(a_sb[wp][:, :], asb16[p][:, :])

            with tc.For_i(0, iters, hint_engines=(mybir.EngineType.PE,)) as i:
                for u in range(unroll):
                    step(u)
                    if u == xh - 1:
                        nc.sync.dma_start(
                            x_stg[0][:, :],
                            x_dram[:, ds((i + 1) * (unroll * 128), xh * 128)],
                        )
                nc.sync.dma_start(
                    x_stg[1][:, :],
                    x_dram[:, ds((i + 1) * (unroll * 128) + xh * 128, xh * 128)],
                )

            # ---- epilogue: logits = h W_d^T + b ; softmax ----
            ps_l = psh[0][0:BL, 0:10]   # loop done; reuse bank
            for kt in range(8):
                lhsT = (stHa[0][:, ds(32 * kt, BL)] if kt < 4
                        else stHb[0][:, ds(32 * (kt - 4), BL)])
                nc.tensor.matmul(
                    ps_l, lhsT, wd_sb[:, ts(kt, 10)],
                    start=(kt == 0), stop=False,
                )
            nc.tensor.matmul(
                ps_l, ones_sb[:, :], bias_sb[:, :], start=False, stop=True
            )
            sm = wpool.tile([BL, 10], FP32, tag="sm")
            nc.scalar.activation(sm[:, :], ps_l, mybir.ActivationFunctionType.Exp)
            ssum = wpool.tile([BL, 1], FP32, tag="ssum")
            nc.vector.reduce_sum(ssum[:, :], sm[:, :], axis=mybir.AxisListType.X)
            srec = wpool.tile([BL, 1], FP32, tag="srec")
            nc.vector.reciprocal(srec[:, :], ssum[:, :])
            nc.vector.tensor_scalar_mul(sm[:, :], sm[:, :], srec[:, :])
            nc.sync.dma_start(out_dram[:, :], sm[:, :])

    nc.compile()
    return nc


_NC_CACHE = {}


def _get_nc(t_steps=T, unroll=UNROLL):
    key = (t_steps, unroll)
    if key not in _NC_CACHE:
        _NC_CACHE[key] = _build_nc(t_steps, unroll)
    return _NC_CACHE[key]


def kernel(inputs, e_x, e_h, e_m, W_x, W_h, W_m, AT, BT, W_dense, b_dense,
           _t_steps=T, _unroll=UNROLL, _trace=False):
    inputs = np.asarray(inputs, np.float32)
    args = [np.asarray(a, np.float32)
            for a in (e_x, e_h, e_m, W_x, W_h, W_m, AT, BT, W_dense, b_dense)]
    e_x, e_h, e_m, W_x, W_h, W_m, AT, BT, W_dense, b_dense = args

    wh, wmh, wmm, xwh, xwm, awh, awm = _build_weights(
        e_x, e_h, e_m, W_x, W_h, W_m, AT, BT
    )
    IP = _perms()
    wd = np.zeros((UNITS + 1, 10), np.float16)
    wd[:UNITS, :] = W_dense.T[IP[:UNITS], :].astype(np.float16)
    wd[UNITS, :] = b_dense.astype(np.float16)

    x = inputs[:_t_steps, :, 0].astype(np.float16)        # (T, B)
    nc = _get_nc(_t_steps, _unroll)
    in_maps = []
    for c in range(NCORES):
        xc = x[:, c * BL:(c + 1) * BL]                    # (T, BL)
        xs4 = np.zeros((NSTRIP, _t_steps + _unroll, 128), np.float16)
        for k in range(NSTRIP):
            xs4[k, :_t_steps, 32 * k : 32 * k + BL] = xc
        xs = np.ascontiguousarray(
            xs4.reshape(NSTRIP, (_t_steps + _unroll) * 128)
        )
        in_maps.append({
            "wh": wh, "wmh": wmh, "wmm": wmm, "xwh": xwh, "xwm": xwm,
            "awh": awh, "awm": awm, "wd": wd, "xs": xs,
        })

    res = run_bass_kernel_spmd(
        nc, in_maps, core_ids=list(range(NCORES)), trace=_trace
    )
    out = np.concatenate([res.results[c]["out"] for c in range(NCORES)], axis=0)
    kernel.last_results = res
    return out.astype(np.float32)
